# revision 28
# baseline (speedup 1.0000x reference)
"""Trainium2 Bass kernel for nn_ExtractPatchesPositionLayer.

Reference semantics: per image b, bilinear-translate the (522,522,1) padded
object by t = -positions[b] (tfa.translate: out(y,x) = img(y+py, x+px),
zero fill outside), then center-crop 5px -> (512,512,1).

The shift is constant per image, so floor/frac of the offset give an integer
window start (A,B) plus four bilinear corner weights c00,c01,c10,c11. The
host extracts each image's integer-aligned 513x513 window (zero-padded at the
borders, row-padded to 514 for even alignment) and casts it to fp16 — after
that every device access pattern is STATIC, so all DMAs are plain HWDGE
copies that spray evenly across all 16 SDMA engines (dynamic-offset DMAs all
serialize on one engine/queue, which was the original 1.4 ms bottleneck).

Blocked layout, halo-free: SBUF partition p holds exactly window rows
4p+1..4p+4 (one contiguous 4-row read per partition, zero redundant HBM
traffic). Output row 4p+k taps window rows 4p+k (c00/c01) and 4p+k+1
(c10/c11); with this staging the c10/c11 taps sit at free offset k*RS and
the c00/c01 taps at (k-1)*RS, all free-dim shifts, evaluated on the
(otherwise idle) tensor engine as 4 accumulating matmuls per 512-wide chunk
with SCALED-IDENTITY stationary weights (lhsT = c*I). For k=0 the c00/c01
taps live in the PREVIOUS partition's last row, which the PE reaches with a
scaled SUPERDIAGONAL lhsT (c*S, S[k,m]=d_{k,m-1} -> out[m]+=c*rhs[m-1]) —
same matmul count. Window row 0 (affects only output row 0 = partition 0,
quadrant-aligned) is DMA'd into partition 0 of a side tile and folded in by
two [1,512] fused mul-adds on DVE during the drain. PSUM accumulates in
fp32; the result is rounded once to fp16 for the store (output HBM traffic
halves; the host upcasts — total rel err ~6e-4, far under the 2e-2 gate).
Sharding: batch 256 -> 32 images x 8 cores, embarrassingly parallel.
"""

from dataclasses import dataclass

import numpy as np

import concourse.bacc as bacc
import concourse.bass as bass
import concourse.mybir as mybir
import concourse.tile as tile
from concourse.bass_utils import run_bass_kernel_spmd

PAD = 5


@dataclass(frozen=True)
class Cfg:
    bpc: int   # images per core
    n: int     # output height/width (512)

    @property
    def win(self):  # window rows/cols actually used
        return self.n + 1

    @property
    def rs(self):   # row stride in the staged window (win padded to even)
        return self.win + 1

    @property
    def rs2(self):  # row stride of the edge-row tile
        return self.win + 3

    @property
    def rpp(self):  # output rows per partition
        return self.n // 128


def build_nc(cfg: Cfg) -> bass.Bass:
    BPC, N, RS, RS2 = cfg.bpc, cfg.n, cfg.rs, cfg.rs2
    K = cfg.rpp                 # 4 output rows per partition
    IMG = N * RS                # elems per staged image body (512*514)
    NN = N * N                  # elems per output image
    f16 = mybir.dt.float16
    f32 = mybir.dt.float32
    mult = mybir.AluOpType.mult
    add = mybir.AluOpType.add

    nc = bacc.Bacc("TRN2", target_bir_lowering=False, debug=False)
    x_d = nc.declare_dram_parameter("x", [BPC, IMG], f16, isOutput=False)
    x2_d = nc.declare_dram_parameter("x2", [BPC, RS2], f16, isOutput=False)
    wm_d = nc.declare_dram_parameter("wm", [128, BPC * 4], f32, isOutput=False)
    id_d = nc.declare_dram_parameter("idm", [128, 256], f16, isOutput=False)
    y_d = nc.declare_dram_parameter("y", [BPC, NN], f16, isOutput=True)

    with tile.TileContext(nc) as tc:
        with (
            tc.tile_pool(name="const", bufs=1) as constp,
            tc.tile_pool(name="win", bufs=3) as winp,
            tc.tile_pool(name="lt", bufs=2) as ltp,
            tc.tile_pool(name="outp", bufs=3) as outp,
            tc.tile_pool(name="ps", bufs=2, space="PSUM") as psp,
        ):
            # constants ride the ACT HWDGE ring so the first window loads
            # on the SP ring start immediately
            wm_sb = constp.tile([128, BPC * 4], f32, tag="wm")
            nc.scalar.dma_start(wm_sb[:], wm_d[:, :])
            # [I | S'] stacked: one mul scales both the identity and the
            # superdiagonal shift matrix by the same corner weight
            id_sb = constp.tile([128, 256], f16, tag="idm")
            nc.scalar.dma_start(id_sb[:], id_d[:, :])

            for b in range(BPC):
                # partition p <- window rows K*p .. K*p+K-1, contiguous
                w = winp.tile([128, K * RS], f16, tag="w")
                nc.sync.dma_start(
                    w[:], bass.AP(x_d, b * IMG, [[K * RS, 128], [1, K * RS]]))
                # window row 0 -> partition 0 of a side tile
                w2 = winp.tile([128, RS2], f16, tag="w2")
                nc.sync.dma_start(
                    w2[0:1, :], bass.AP(x2_d, b * RS2, [[RS2, 1], [1, RS2]]))

                # stationary weights: c*[I|S'] for c00/c01 (one mul builds
                # both), c*I for c10/c11
                cw = [wm_sb[:, 4 * b + ij: 4 * b + ij + 1] for ij in range(4)]
                lt0x = ltp.tile([128, 256], f16, tag="lt0x")
                nc.vector.tensor_scalar_mul(lt0x[:], id_sb[:], cw[0])
                lt1x = ltp.tile([128, 256], f16, tag="lt1x")
                nc.vector.tensor_scalar_mul(lt1x[:], id_sb[:], cw[1])
                lt2 = ltp.tile([128, 128], f16, tag="lt2")
                nc.vector.tensor_scalar_mul(lt2[:], id_sb[:, 0:128], cw[2])
                lt3 = ltp.tile([128, 128], f16, tag="lt3")
                nc.vector.tensor_scalar_mul(lt3[:], id_sb[:, 0:128], cw[3])
                lts = [lt0x, lt1x, lt2, lt3]
                ls00 = lt0x[:, 128:256]
                ls01 = lt1x[:, 128:256]

                # 4 taps x 4 chunks = 16 accumulating matmuls, emitted
                # chunk-by-chunk so each chunk's PSUM drain can overlap the
                # remaining chunks' matmuls (the PE reloads weights per
                # matmul either way)
                ps = psp.tile([128, K * N], f32, tag="ps")

                def mm(lhsT, k, off, start, stop):
                    nc.tensor.matmul(
                        out=ps[:, k * N:(k + 1) * N], lhsT=lhsT,
                        rhs=w[:, off: off + N], start=start, stop=stop)

                for k in range(K):
                    if k == 0:  # c00/c01 via the superdiagonal shift
                        mm(ls00, 0, (K - 1) * RS, True, False)
                        mm(ls01, 0, (K - 1) * RS + 1, False, False)
                    else:       # c00/c01: previous row block, offset k-1
                        mm(lt0x[:, 0:128], k, (k - 1) * RS, True, False)
                        mm(lt1x[:, 0:128], k, (k - 1) * RS + 1, False, False)
                    mm(lt2[:], k, k * RS, False, False)          # c10
                    mm(lt3[:], k, k * RS + 1, False, True)       # c11
                    if k == 0:
                        # fold window row 0 into output row 0 in PSUM while
                        # chunks 1-3 are still on the PE (keeps the patches
                        # off the store's dependency chain)
                        pfirst = ps[0:1, 0:N]
                        nc.vector.scalar_tensor_tensor(
                            pfirst, w2[0:1, 0:N],
                            wm_sb[0:1, 4 * b + 0: 4 * b + 1], pfirst,
                            mult, add)
                        nc.vector.scalar_tensor_tensor(
                            pfirst, w2[0:1, 1:1 + N],
                            wm_sb[0:1, 4 * b + 1: 4 * b + 2], pfirst,
                            mult, add)

                # PSUM -> SBUF with a single fp32->fp16 rounding, split
                # across DVE and ACT
                o = outp.tile([128, K * N], f16, tag="o")
                cut = 768  # DVE is the busier engine; ACT takes the bigger part
                nc.vector.tensor_copy(o[:, 0:cut], ps[:, 0:cut])
                nc.scalar.copy(o[:, cut:], ps[:, cut:])
                # partition p -> output rows K*p .. K*p+K-1 (4 KB contiguous)
                nc.scalar.dma_start(
                    bass.AP(y_d, b * NN, [[K * N, 128], [1, K * N]]), o[:])
    nc.compile()
    return nc


def host_prep(padded: np.ndarray, positions: np.ndarray, n_cores: int):
    """Shard + stage integer-aligned fp16 windows.

    padded: (B, npad, npad) f32, positions: (B, 2)."""
    B, npad, _ = padded.shape
    n = npad - 2 * PAD
    cfg = Cfg(bpc=B // n_cores, n=n)
    win, rs, rs2 = cfg.win, cfg.rs, cfg.rs2

    px = positions[:, 0].astype(np.float64)
    py = positions[:, 1].astype(np.float64)
    fy = np.floor(py)
    fx = np.floor(px)
    ay = (PAD + fy).astype(np.int64)
    ax = (PAD + fx).astype(np.int64)
    wy = (py - fy).astype(np.float32)
    wx = (px - fx).astype(np.float32)

    xw = np.zeros((B, win, rs), dtype=np.float16)
    for b in range(B):
        r0 = max(int(ay[b]), 0)
        r1 = min(int(ay[b]) + win, npad)
        c0 = max(int(ax[b]), 0)
        c1 = min(int(ax[b]) + win, npad)
        if r1 > r0 and c1 > c0:
            xw[b, r0 - ay[b]:r1 - ay[b], c0 - ax[b]:c1 - ax[b]] = \
                padded[b, r0:r1, c0:c1]
    x2 = np.zeros((B, rs2), dtype=np.float16)
    x2[:, 0:win] = xw[:, 0, 0:win]             # window row 0

    bpc = cfg.bpc
    idm = np.concatenate(
        [np.eye(128, dtype=np.float16),
         np.eye(128, k=1, dtype=np.float16)], axis=1)  # [I | S']
    in_maps = []
    for cidx in range(n_cores):
        sl = slice(cidx * bpc, (cidx + 1) * bpc)
        wmat = np.empty((128, bpc * 4), dtype=np.float32)
        wmat[:, 0::4] = ((1 - wy[sl]) * (1 - wx[sl]))[None, :]  # c00: no shift
        wmat[:, 1::4] = ((1 - wy[sl]) * wx[sl])[None, :]        # c01: +1 col
        wmat[:, 2::4] = (wy[sl] * (1 - wx[sl]))[None, :]        # c10: +1 row
        wmat[:, 3::4] = (wy[sl] * wx[sl])[None, :]              # c11: both
        in_maps.append({
            "x": xw[sl, 1:win, :].reshape(bpc, n * rs),
            "x2": x2[sl],
            "wm": wmat,
            "idm": idm,
        })
    return cfg, in_maps


N_CORES = 8
_nc_cache: dict = {}


def kernel(padded_obj: np.ndarray, positions: np.ndarray) -> np.ndarray:
    padded_obj = np.asarray(padded_obj)
    positions = np.asarray(positions)
    B, npad, _, C = padded_obj.shape
    cfg, in_maps = host_prep(
        padded_obj.reshape(B, npad, npad).astype(np.float32, copy=False),
        positions, N_CORES)

    nc = _nc_cache.get(cfg)
    if nc is None:
        nc = build_nc(cfg)
        _nc_cache[cfg] = nc

    res = run_bass_kernel_spmd(nc, in_maps, core_ids=list(range(N_CORES)))
    out = np.concatenate([r["y"] for r in res.results], axis=0)
    return out.reshape(B, cfg.n, cfg.n, 1).astype(np.float32)


# revision 29
# speedup vs baseline: 1.4260x; 1.4260x over previous
"""Trainium2 Bass kernel for nn_ExtractPatchesPositionLayer.

Reference semantics: per image b, bilinear-translate the (522,522,1) padded
object by t = -positions[b] (tfa.translate: out(y,x) = img(y+py, x+px),
zero fill outside), then center-crop 5px -> (512,512,1).

The shift is constant per image, so floor/frac of the offset give an integer
window start (A,B) plus four bilinear corner weights c00,c01,c10,c11. The
host extracts each image's integer-aligned 513x513 window (zero-padded at the
borders, row-padded to 514 for even alignment) and casts it to fp16 — after
that every device access pattern is STATIC, so all DMAs are plain HWDGE
copies that spray evenly across all 16 SDMA engines (dynamic-offset DMAs all
serialize on one engine/queue, which was the original 1.4 ms bottleneck).

Blocked layout, halo-free: SBUF partition p holds exactly window rows
4p+1..4p+4 (one contiguous 4-row read per partition, zero redundant HBM
traffic). Output row 4p+k taps window rows 4p+k (c00/c01) and 4p+k+1
(c10/c11); with this staging the c10/c11 taps sit at free offset k*RS and
the c00/c01 taps at (k-1)*RS, all free-dim shifts, evaluated on the
(otherwise idle) tensor engine as 4 accumulating matmuls per 512-wide chunk
with SCALED-IDENTITY stationary weights (lhsT = c*I). For k=0 the c00/c01
taps live in the PREVIOUS partition's last row, which the PE reaches with a
scaled SUPERDIAGONAL lhsT (c*S, S[k,m]=d_{k,m-1} -> out[m]+=c*rhs[m-1]) —
same matmul count. Window row 0 (affects only output row 0 = partition 0,
quadrant-aligned) is DMA'd into partition 0 of a side tile and folded in by
two [1,512] fused mul-adds on DVE during the drain. PSUM accumulates in
fp32; the result is rounded once to fp16 for the store (output HBM traffic
halves; the host upcasts — total rel err ~6e-4, far under the 2e-2 gate).
Sharding: batch 256 -> 32 images x 8 cores, embarrassingly parallel.
"""

from dataclasses import dataclass

import numpy as np

import concourse.bacc as bacc
import concourse.bass as bass
import concourse.mybir as mybir
import concourse.tile as tile
from concourse.bass_utils import run_bass_kernel_spmd

PAD = 5


@dataclass(frozen=True)
class Cfg:
    bpc: int   # images per core
    n: int     # output height/width (512)

    @property
    def win(self):  # window rows/cols actually used
        return self.n + 1

    @property
    def rs(self):   # row stride in the staged window (win padded to even)
        return self.win + 1

    @property
    def rs2(self):  # row stride of the edge-row tile
        return self.win + 3

    @property
    def rpp(self):  # output rows per partition
        return self.n // 128


def build_nc(cfg: Cfg) -> bass.Bass:
    BPC, N, RS, RS2 = cfg.bpc, cfg.n, cfg.rs, cfg.rs2
    K = cfg.rpp                 # 4 output rows per partition
    IMG = N * RS                # elems per staged image body (512*514)
    NN = N * N                  # elems per output image
    f16 = mybir.dt.float16
    f32 = mybir.dt.float32
    mult = mybir.AluOpType.mult
    add = mybir.AluOpType.add

    nc = bacc.Bacc("TRN2", target_bir_lowering=False, debug=False)
    x_d = nc.declare_dram_parameter("x", [BPC, IMG], f16, isOutput=False)
    x2_d = nc.declare_dram_parameter("x2", [BPC, RS2], f16, isOutput=False)
    wm_d = nc.declare_dram_parameter("wm", [128, BPC * 4], f32, isOutput=False)
    id_d = nc.declare_dram_parameter("idm", [128, 256], f16, isOutput=False)
    y_d = nc.declare_dram_parameter("y", [BPC, NN], f16, isOutput=True)

    with tile.TileContext(nc) as tc:
        with (
            tc.tile_pool(name="const", bufs=1) as constp,
            tc.tile_pool(name="win", bufs=3) as winp,
            tc.tile_pool(name="lt", bufs=2) as ltp,
            tc.tile_pool(name="outp", bufs=3) as outp,
            tc.tile_pool(name="ps", bufs=2, space="PSUM") as psp,
        ):
            # constants ride the ACT HWDGE ring so the first window loads
            # on the SP ring start immediately
            wm_sb = constp.tile([128, BPC * 4], f32, tag="wm")
            nc.scalar.dma_start(wm_sb[:], wm_d[:, :])
            # [I | S'] stacked: one mul scales both the identity and the
            # superdiagonal shift matrix by the same corner weight
            id_sb = constp.tile([128, 256], f16, tag="idm")
            nc.scalar.dma_start(id_sb[:], id_d[:, :])

            for b in range(BPC):
                # partition p <- window rows K*p .. K*p+K-1, contiguous
                w = winp.tile([128, K * RS], f16, tag="w")
                nc.sync.dma_start(
                    w[:], bass.AP(x_d, b * IMG, [[K * RS, 128], [1, K * RS]]))
                # window row 0 -> partition 0 of a side tile
                w2 = winp.tile([128, RS2], f16, tag="w2")
                nc.sync.dma_start(
                    w2[0:1, :], bass.AP(x2_d, b * RS2, [[RS2, 1], [1, RS2]]))

                # stationary weights: c*[I|S'] for c00/c01 (one mul builds
                # both), c*I for c10/c11
                cw = [wm_sb[:, 4 * b + ij: 4 * b + ij + 1] for ij in range(4)]
                lt0x = ltp.tile([128, 256], f16, tag="lt0x")
                nc.vector.tensor_scalar_mul(lt0x[:], id_sb[:], cw[0])
                lt1x = ltp.tile([128, 256], f16, tag="lt1x")
                nc.vector.tensor_scalar_mul(lt1x[:], id_sb[:], cw[1])
                lt2 = ltp.tile([128, 128], f16, tag="lt2")
                nc.vector.tensor_scalar_mul(lt2[:], id_sb[:, 0:128], cw[2])
                lt3 = ltp.tile([128, 128], f16, tag="lt3")
                nc.vector.tensor_scalar_mul(lt3[:], id_sb[:, 0:128], cw[3])
                lts = [lt0x, lt1x, lt2, lt3]
                ls00 = lt0x[:, 128:256]
                ls01 = lt1x[:, 128:256]

                # 4 taps x 4 chunks = 16 accumulating matmuls, emitted
                # chunk-by-chunk so each chunk's PSUM drain can overlap the
                # remaining chunks' matmuls (the PE reloads weights per
                # matmul either way)
                ps = psp.tile([128, K * N], f32, tag="ps")

                def mm(lhsT, k, off, start, stop):
                    nc.tensor.matmul(
                        out=ps[:, k * N:(k + 1) * N], lhsT=lhsT,
                        rhs=w[:, off: off + N], start=start, stop=stop)

                for k in range(K):
                    if k == 0:  # c00/c01 via the superdiagonal shift
                        mm(ls00, 0, (K - 1) * RS, True, False)
                        mm(ls01, 0, (K - 1) * RS + 1, False, False)
                    else:       # c00/c01: previous row block, offset k-1
                        mm(lt0x[:, 0:128], k, (k - 1) * RS, True, False)
                        mm(lt1x[:, 0:128], k, (k - 1) * RS + 1, False, False)
                    mm(lt2[:], k, k * RS, False, False)          # c10
                    mm(lt3[:], k, k * RS + 1, False, True)       # c11

                # PSUM -> SBUF with a single fp32->fp16 rounding, split
                # across DVE and ACT
                o = outp.tile([128, K * N], f16, tag="o")
                cut = 768  # DVE is the busier engine; ACT takes the bigger part
                nc.vector.tensor_copy(o[:, 0:cut], ps[:, 0:cut])
                nc.scalar.copy(o[:, cut:], ps[:, cut:])
                # fold window row 0 into output row 0 (partition 0,
                # chunk 0): two in-place [1,512] fused mul-adds on DVE
                first = o[0:1, 0:N]
                nc.vector.scalar_tensor_tensor(
                    first, w2[0:1, 0:N],
                    wm_sb[0:1, 4 * b + 0: 4 * b + 1], first, mult, add)
                nc.vector.scalar_tensor_tensor(
                    first, w2[0:1, 1:1 + N],
                    wm_sb[0:1, 4 * b + 1: 4 * b + 2], first, mult, add)
                # partition p -> output rows K*p .. K*p+K-1 (4 KB contiguous)
                nc.scalar.dma_start(
                    bass.AP(y_d, b * NN, [[K * N, 128], [1, K * N]]), o[:])
    nc.compile()
    return nc


def host_prep(padded: np.ndarray, positions: np.ndarray, n_cores: int):
    """Shard + stage integer-aligned fp16 windows.

    padded: (B, npad, npad) f32, positions: (B, 2)."""
    B, npad, _ = padded.shape
    n = npad - 2 * PAD
    cfg = Cfg(bpc=B // n_cores, n=n)
    win, rs, rs2 = cfg.win, cfg.rs, cfg.rs2

    px = positions[:, 0].astype(np.float64)
    py = positions[:, 1].astype(np.float64)
    fy = np.floor(py)
    fx = np.floor(px)
    ay = (PAD + fy).astype(np.int64)
    ax = (PAD + fx).astype(np.int64)
    wy = (py - fy).astype(np.float32)
    wx = (px - fx).astype(np.float32)

    xw = np.zeros((B, win, rs), dtype=np.float16)
    for b in range(B):
        r0 = max(int(ay[b]), 0)
        r1 = min(int(ay[b]) + win, npad)
        c0 = max(int(ax[b]), 0)
        c1 = min(int(ax[b]) + win, npad)
        if r1 > r0 and c1 > c0:
            xw[b, r0 - ay[b]:r1 - ay[b], c0 - ax[b]:c1 - ax[b]] = \
                padded[b, r0:r1, c0:c1]
    x2 = np.zeros((B, rs2), dtype=np.float16)
    x2[:, 0:win] = xw[:, 0, 0:win]             # window row 0

    bpc = cfg.bpc
    idm = np.concatenate(
        [np.eye(128, dtype=np.float16),
         np.eye(128, k=1, dtype=np.float16)], axis=1)  # [I | S']
    in_maps = []
    for cidx in range(n_cores):
        sl = slice(cidx * bpc, (cidx + 1) * bpc)
        wmat = np.empty((128, bpc * 4), dtype=np.float32)
        wmat[:, 0::4] = ((1 - wy[sl]) * (1 - wx[sl]))[None, :]  # c00: no shift
        wmat[:, 1::4] = ((1 - wy[sl]) * wx[sl])[None, :]        # c01: +1 col
        wmat[:, 2::4] = (wy[sl] * (1 - wx[sl]))[None, :]        # c10: +1 row
        wmat[:, 3::4] = (wy[sl] * wx[sl])[None, :]              # c11: both
        in_maps.append({
            "x": xw[sl, 1:win, :].reshape(bpc, n * rs),
            "x2": x2[sl],
            "wm": wmat,
            "idm": idm,
        })
    return cfg, in_maps


N_CORES = 8
_nc_cache: dict = {}


def kernel(padded_obj: np.ndarray, positions: np.ndarray) -> np.ndarray:
    padded_obj = np.asarray(padded_obj)
    positions = np.asarray(positions)
    B, npad, _, C = padded_obj.shape
    cfg, in_maps = host_prep(
        padded_obj.reshape(B, npad, npad).astype(np.float32, copy=False),
        positions, N_CORES)

    nc = _nc_cache.get(cfg)
    if nc is None:
        nc = build_nc(cfg)
        _nc_cache[cfg] = nc

    res = run_bass_kernel_spmd(nc, in_maps, core_ids=list(range(N_CORES)))
    out = np.concatenate([r["y"] for r in res.results], axis=0)
    return out.reshape(B, cfg.n, cfg.n, 1).astype(np.float32)
